# revision 21
# baseline (speedup 1.0000x reference)
"""Trainium2 Bass kernel: 4-layer dense transformer (Megatron TP over 8 NeuronCores).

Sharding (per sharding hint):
  - qkv_w / up_w sharded along output dim, out_w / down_w along input dim.
  - Heads: 16 heads / 8 cores = 2 heads per core.
  - vocab_w sharded along vocab dim (4000 cols/core); host concatenates logits.
  - Norms + residual replicated; deltas AllReduced (bf16) after out-proj / down-proj.

Device layout strategy:
  - Residual `emb` kept in natural [s, e] layout (bf16), s-tiles of 128 on partitions.
  - Matmul inputs need e (contraction) on partitions -> normed activations are
    transposed via PE transpose; the rmsnorm scale is folded into the transpose by
    replacing the identity with diag(scale).
  - Attention computes S^T = K @ Q^T per head (k on partitions), exp via ACT,
    causal mask via gpsimd.affine_select on diagonal blocks, AV via
    matmul(lhsT=P^T, rhs=[V|1]) which yields O and the softmax denominator in
    one accumulation group.
"""

import os
import sys

if "/opt/trn_rl_repo" not in sys.path:
    sys.path.insert(0, "/opt/trn_rl_repo")

import math
from dataclasses import dataclass

import numpy as np
import ml_dtypes

import concourse.bass as bass
import concourse.bacc as bacc
import concourse.mybir as mybir
import concourse.tile as tile
from concourse.bass import IndirectOffsetOnAxis
from concourse.bass_utils import run_bass_kernel_spmd
from concourse.masks import make_identity

F32 = mybir.dt.float32
BF16 = mybir.dt.bfloat16
I32 = mybir.dt.int32
AF = mybir.ActivationFunctionType
OP = mybir.AluOpType

P = 128


@dataclass(frozen=True)
class Cfg:
    S: int = 2048        # sequence
    E: int = 1024        # embed
    D: int = 64          # head dim
    HC: int = 2          # heads per core
    HIDC: int = 512      # hidden shard per core (of 4096 / 8)
    VC: int = 4000       # vocab shard per core
    L: int = 4           # layers
    n_cores: int = 8
    V_ROWS: int = 32000  # embedding table rows
    eps: float = 1e-5

    @property
    def ST(self):
        return self.S // P

    @property
    def ET(self):
        return self.E // P

    @property
    def HT(self):
        return self.HIDC // P

    @property
    def QKVC(self):
        return 3 * self.HC * self.D

    @property
    def QC(self):
        return min(512, self.S)  # q chunk for attention

    @property
    def VCHUNK(self):
        for c in (500, 512, 256, 128):
            if self.VC % c == 0 and c <= 512:
                return c
        return self.VC


def build_kernel(cfg: Cfg):
    """Build the SPMD Bass graph. Returns the nc module."""
    c = cfg
    ST, ET, HT = c.ST, c.ET, c.HT
    D, HC = c.D, c.HC
    D2 = D // 2
    EG = min(4, ET)  # transpose group size (transposes per psum tile)

    nc = bacc.Bacc(
        "TRN2", target_bir_lowering=False, debug=False, num_devices=cfg.n_cores
    )

    # ---- DRAM parameters (per-core shards; same names in in_maps) ----
    idx_d = nc.declare_dram_parameter("idx", [P, ST], I32, isOutput=False)
    table_d = nc.declare_dram_parameter("table", [c.V_ROWS, c.E], BF16, isOutput=False)
    qkvw_d = nc.declare_dram_parameter("qkvw", [c.L, c.E, c.QKVC], BF16, isOutput=False)
    outw_d = nc.declare_dram_parameter("outw", [c.L, HC * D, c.E], BF16, isOutput=False)
    upw_d = nc.declare_dram_parameter("upw", [c.L, c.E, 2 * c.HIDC], BF16, isOutput=False)
    downw_d = nc.declare_dram_parameter("downw", [c.L, c.HIDC, c.E], BF16, isOutput=False)
    vocw_d = nc.declare_dram_parameter("vocw", [c.E, c.VC], BF16, isOutput=False)
    cosq_d = nc.declare_dram_parameter("cosq", [P, ST * D], BF16, isOutput=False)
    sinq_d = nc.declare_dram_parameter("sinq", [P, ST * D], BF16, isOutput=False)
    cosk_d = nc.declare_dram_parameter("cosk", [P, ST * D], BF16, isOutput=False)
    sink_d = nc.declare_dram_parameter("sink", [P, ST * D], BF16, isOutput=False)
    out_d = nc.declare_dram_parameter("out", [c.S, c.VC], F32, isOutput=True)

    from contextlib import ExitStack

    with tile.TileContext(nc) as tc, ExitStack() as es:
        const = es.enter_context(tc.tile_pool(name="const", bufs=1))
        wpool = es.enter_context(tc.tile_pool(name="w", bufs=1))
        act = es.enter_context(tc.tile_pool(name="act", bufs=1))
        act2 = es.enter_context(tc.tile_pool(name="act2", bufs=1))
        work = es.enter_context(tc.tile_pool(name="work", bufs=2))
        psum = es.enter_context(tc.tile_pool(name="psum", bufs=4, space="PSUM"))
        psum_tr = es.enter_context(tc.tile_pool(name="psumtr", bufs=2, space="PSUM"))
        dram = es.enter_context(tc.tile_pool(name="dram", bufs=4, space="DRAM"))
        if True:
            # ---- constants ----
            ident = const.tile([P, P], BF16, tag="ident")
            make_identity(nc, ident[:])
            rope_tiles = {}
            for name, d in (("cosq", cosq_d), ("sinq", sinq_d),
                            ("cosk", cosk_d), ("sink", sink_d)):
                t = const.tile([P, ST, D], BF16, tag=name)
                nc.sync.dma_start(t[:], d[:].rearrange("p (st d) -> p st d", d=D))
                rope_tiles[name] = t
            idx_sb = const.tile([P, ST], I32, tag="idx")
            nc.sync.dma_start(idx_sb[:], idx_d[:])

            # ---- residual (bf16, natural [s-tile, e]) via embedding gather ----
            emb = act.tile([P, ST, c.E], BF16, tag="emb")
            for st in range(ST):
                nc.gpsimd.indirect_dma_start(
                    out=emb[:, st, :],
                    out_offset=None,
                    in_=table_d[:],
                    in_offset=IndirectOffsetOnAxis(ap=idx_sb[:, st : st + 1], axis=0),
                )

            def rmsnorm_T():
                """rmsnorm(emb) transposed -> [P, ET, S] bf16 (e on partitions)."""
                ssq = work.tile([P, ST], F32, tag="ssq")
                for st in range(ST):
                    scr = work.tile([P, c.E], BF16, tag="sqscr")
                    nc.scalar.activation(
                        out=scr[:], in_=emb[:, st, :], func=AF.Square,
                        accum_out=ssq[:, st : st + 1],
                    )
                ms = work.tile([P, ST], F32, tag="ms")
                nc.vector.tensor_scalar(
                    ms[:], ssq[:], 1.0 / c.E, c.eps, OP.mult, OP.add
                )
                rms = work.tile([P, ST], F32, tag="rms")
                nc.scalar.activation(out=rms[:], in_=ms[:], func=AF.Sqrt)
                scale = work.tile([P, ST], F32, tag="scale")
                nc.vector.reciprocal(out=scale[:], in_=rms[:])
                scale_b = work.tile([P, ST], BF16, tag="scaleb")
                nc.vector.tensor_copy(out=scale_b[:], in_=scale[:])

                normT = act2.tile([P, ET, c.S], BF16, tag="big")
                for st in range(ST):
                    diag = work.tile([P, P], BF16, tag="diag")
                    nc.gpsimd.affine_select(
                        out=diag[:],
                        in_=scale_b[:, st : st + 1].to_broadcast((P, P)),
                        pattern=[[-1, P]],
                        compare_op=OP.is_equal,
                        fill=0.0,
                        base=0,
                        channel_multiplier=1,
                    )
                    for eg in range(ET // EG):
                        ptr = psum_tr.tile([P, 512], F32, tag="tr", name="tr")[:, : EG * P]
                        for j in range(EG):
                            ee = eg * EG + j
                            nc.tensor.matmul(
                                ptr[:, j * P : (j + 1) * P],
                                lhsT=emb[:, st, ee * P : (ee + 1) * P],
                                rhs=diag[:],
                                start=True,
                                stop=True,
                            )
                        nc.vector.tensor_copy(
                            out=normT[:, eg * EG : (eg + 1) * EG,
                                      st * P : (st + 1) * P],
                            in_=ptr.rearrange("p (g q) -> p g q", g=EG),
                        )
                return normT

            def allreduce_add_delta(delta):
                """delta: [P, ST, E] bf16 partial -> AllReduce -> emb += result."""
                arin = dram.tile([c.S, c.E], BF16, tag="arin")
                arout = dram.tile([c.S, c.E], BF16, tag="arout", addr_space="Shared")
                nc.sync.dma_start(
                    arin[:].rearrange("(st p) e -> p st e", p=P), delta[:]
                )
                nc.gpsimd.collective_compute(
                    "AllReduce",
                    OP.add,
                    replica_groups=[list(range(c.n_cores))],
                    ins=[arin[:].opt()],
                    outs=[arout[:].opt()],
                )
                gath = act2.tile([P, ST, c.E], BF16, tag="big")
                nc.sync.dma_start(
                    gath[:], arout[:].rearrange("(st p) e -> p st e", p=P)
                )
                nc.vector.tensor_tensor(
                    out=emb[:], in0=emb[:], in1=gath[:], op=OP.add
                )

            for l in range(c.L):
                # ---- layer weights (bf16, e/h on partitions) ----
                qkvw = wpool.tile([P, ET, c.QKVC], BF16, tag="qkvw", bufs=2)
                nc.sync.dma_start(
                    qkvw[:], qkvw_d[l].rearrange("(ko p) n -> p ko n", p=P)
                )
                outw = wpool.tile([P, c.E], BF16, tag="outw", bufs=2)
                nc.sync.dma_start(outw[:], outw_d[l])
                upw = wpool.tile([P, ET, 2 * c.HIDC], BF16, tag="upw")
                nc.sync.dma_start(
                    upw[:], upw_d[l].rearrange("(ko p) n -> p ko n", p=P)
                )
                downw = wpool.tile([P, HT, c.E], BF16, tag="downw")
                nc.sync.dma_start(
                    downw[:], downw_d[l].rearrange("(ko p) n -> p ko n", p=P)
                )

                # ======== attention half ========
                normT = rmsnorm_T()

                # qkv (natural [s, 3*HC*D])
                qkvn = act.tile([P, ST, c.QKVC], BF16, tag="qg")
                for st in range(ST):
                    pq = psum.tile([P, 512], F32, tag="mm", name="mm")[:, : c.QKVC]
                    for kk in range(ET):
                        nc.tensor.matmul(
                            pq,
                            lhsT=normT[:, kk, st * P : (st + 1) * P],
                            rhs=qkvw[:, kk, :],
                            start=(kk == 0),
                            stop=(kk == ET - 1),
                        )
                    nc.vector.tensor_copy(out=qkvn[:, st, :], in_=pq)

                # RoPE (free-dim half swap; q also pre-scaled by 1/sqrt(D) via consts)
                rq = act.tile([P, ST, HC * D], BF16, tag="ra")
                rk = act.tile([P, ST, HC * D], BF16, tag="rb")
                for (src0, dst, cosw, sinw) in (
                    (0, rq, rope_tiles["cosq"], rope_tiles["sinq"]),
                    (HC * D, rk, rope_tiles["cosk"], rope_tiles["sink"]),
                ):
                    for h in range(HC):
                        s0 = src0 + h * D
                        x = qkvn[:, :, s0 : s0 + D]
                        o = dst[:, :, h * D : (h + 1) * D]
                        tmp = work.tile([P, ST, D], BF16, tag="ropetmp")
                        nc.vector.tensor_tensor(
                            out=tmp[:, :, 0:D2],
                            in0=qkvn[:, :, s0 + D2 : s0 + D],
                            in1=sinw[:, :, 0:D2],
                            op=OP.mult,
                        )
                        nc.vector.tensor_tensor(
                            out=tmp[:, :, D2:D],
                            in0=qkvn[:, :, s0 : s0 + D2],
                            in1=sinw[:, :, D2:D],
                            op=OP.mult,
                        )
                        nc.vector.tensor_tensor(out=o, in0=x, in1=cosw[:], op=OP.mult)
                        nc.vector.tensor_tensor(out=o, in0=o, in1=tmp[:], op=OP.add)

                # V augmented with ones column per head: [.., h*(D+1)+64] = 1
                vaug = act.tile([P, ST, HC * (D + 1)], BF16, tag="vaug")
                nc.gpsimd.memset(vaug[:], 1.0)
                for h in range(HC):
                    nc.vector.tensor_copy(
                        out=vaug[:, :, h * (D + 1) : h * (D + 1) + D],
                        in_=qkvn[:, :, 2 * HC * D + h * D : 2 * HC * D + (h + 1) * D],
                    )

                # transpose rq/rk -> [hd, s] (both heads stacked on partitions)
                rqT = act.tile([P, c.S], BF16, tag="rqT")
                rkT = act.tile([P, c.S], BF16, tag="rkT")
                for st in range(ST):
                    ptr = psum_tr.tile([P, 512], F32, tag="tr", name="tr")[:, : 2 * P]
                    nc.tensor.matmul(ptr[:, 0:P], lhsT=rq[:, st, :], rhs=ident[:],
                                     start=True, stop=True)
                    nc.tensor.matmul(ptr[:, P : 2 * P], lhsT=rk[:, st, :],
                                     rhs=ident[:], start=True, stop=True)
                    nc.vector.tensor_copy(
                        out=rqT[:, st * P : (st + 1) * P], in_=ptr[:, 0:P]
                    )
                    nc.vector.tensor_copy(
                        out=rkT[:, st * P : (st + 1) * P], in_=ptr[:, P : 2 * P]
                    )

                # scores + AV per head, causal, chunked over q
                atto = act.tile([P, ST, HC * D], BF16, tag="ra")
                n_chunks = c.S // c.QC
                for h in range(HC):
                    hp = slice(h * D, (h + 1) * D)
                    for ch in range(n_chunks):
                        q0 = ch * c.QC
                        nk = (q0 + c.QC) // P
                        PT = act2.tile([P, ST, c.QC], BF16, tag="big")
                        for kt in range(nk):
                            ps = psum.tile([P, 512], F32, tag="mm", name="mm")[:, : c.QC]
                            nc.tensor.matmul(
                                ps,
                                lhsT=rkT[hp, kt * P : (kt + 1) * P],
                                rhs=rqT[hp, q0 : q0 + c.QC],
                                start=True,
                                stop=True,
                            )
                            nc.scalar.activation(
                                out=PT[:, kt, :], in_=ps, func=AF.Exp
                            )
                            if (kt + 1) * P > q0:  # diagonal block: causal mask
                                nc.gpsimd.affine_select(
                                    out=PT[:, kt, :],
                                    in_=PT[:, kt, :],
                                    pattern=[[1, c.QC]],
                                    compare_op=OP.is_ge,
                                    fill=0.0,
                                    base=q0 - kt * P,
                                    channel_multiplier=-1,
                                )
                        for qs in range(c.QC // P):
                            po = psum.tile([P, 512], F32, tag="mm", name="mm")[:, : D + 1]
                            for kt in range(nk):
                                nc.tensor.matmul(
                                    po,
                                    lhsT=PT[:, kt, qs * P : (qs + 1) * P],
                                    rhs=vaug[:, kt, h * (D + 1) : (h + 1) * (D + 1)],
                                    start=(kt == 0),
                                    stop=(kt == nk - 1),
                                )
                            st_out = (q0 // P) + qs
                            rec = work.tile([P, 1], F32, tag="rec")
                            nc.vector.reciprocal(out=rec[:], in_=po[:, D : D + 1])
                            nc.vector.tensor_scalar_mul(
                                atto[:, st_out, hp], po[:, 0:D], rec[:]
                            )

                # transpose attn out -> [hd, s]
                attoT = act.tile([P, c.S], BF16, tag="rb")
                for st in range(ST):
                    ptr = psum_tr.tile([P, 512], F32, tag="tr", name="tr")[:, :P]
                    nc.tensor.matmul(ptr, lhsT=atto[:, st, :], rhs=ident[:],
                                     start=True, stop=True)
                    nc.vector.tensor_copy(
                        out=attoT[:, st * P : (st + 1) * P], in_=ptr
                    )

                # out-proj (partial) -> delta, AllReduce, residual add
                delta = act2.tile([P, ST, c.E], BF16, tag="big")
                EW = min(512, c.E)
                for st in range(ST):
                    for he in range(c.E // EW):
                        pd = psum.tile([P, 512], F32, tag="mm", name="mm")[:, :EW]
                        nc.tensor.matmul(
                            pd,
                            lhsT=attoT[:, st * P : (st + 1) * P],
                            rhs=outw[:, he * EW : (he + 1) * EW],
                            start=True,
                            stop=True,
                        )
                        nc.vector.tensor_copy(
                            out=delta[:, st, he * EW : (he + 1) * EW], in_=pd
                        )
                allreduce_add_delta(delta)

                # ======== FFN half ========
                normT = rmsnorm_T()
                gsil = act.tile([P, HT, c.S], BF16, tag="qg")
                hT = act.tile([P, HT, c.S], BF16, tag="hT")
                n_sc = c.S // 512 if c.S >= 512 else 1
                scw = c.S // n_sc
                for ct in range(2 * HT):
                    for sc in range(n_sc):
                        pu = psum.tile([P, 512], F32, tag="mm", name="mm")[:, :scw]
                        for kk in range(ET):
                            nc.tensor.matmul(
                                pu,
                                lhsT=upw[:, kk, ct * P : (ct + 1) * P],
                                rhs=normT[:, kk, sc * scw : (sc + 1) * scw],
                                start=(kk == 0),
                                stop=(kk == ET - 1),
                            )
                        if ct < HT:  # gate tiles: gsil = g * sigmoid(g)
                            sg = work.tile([P, scw], BF16, tag="sg")
                            nc.scalar.activation(out=sg[:], in_=pu, func=AF.Sigmoid)
                            nc.vector.tensor_tensor(
                                out=gsil[:, ct, sc * scw : (sc + 1) * scw],
                                in0=pu,
                                in1=sg[:],
                                op=OP.mult,
                            )
                        else:  # up tiles: h = up * gsil
                            nc.vector.tensor_tensor(
                                out=hT[:, ct - HT, sc * scw : (sc + 1) * scw],
                                in0=pu,
                                in1=gsil[:, ct - HT, sc * scw : (sc + 1) * scw],
                                op=OP.mult,
                            )

                delta = act2.tile([P, ST, c.E], BF16, tag="big")
                for st in range(ST):
                    for he in range(c.E // EW):
                        pd = psum.tile([P, 512], F32, tag="mm", name="mm")[:, :EW]
                        for kt in range(HT):
                            nc.tensor.matmul(
                                pd,
                                lhsT=hT[:, kt, st * P : (st + 1) * P],
                                rhs=downw[:, kt, he * EW : (he + 1) * EW],
                                start=(kt == 0),
                                stop=(kt == HT - 1),
                            )
                        nc.vector.tensor_copy(
                            out=delta[:, st, he * EW : (he + 1) * EW], in_=pd
                        )
                allreduce_add_delta(delta)

            # ======== final norm + logits ========
            normT = rmsnorm_T()
            for nn in range(c.VC // c.VCHUNK):
                vw = wpool.tile([P, ET, c.VCHUNK], BF16, tag="vocw")
                nc.sync.dma_start(
                    vw[:],
                    vocw_d[:, nn * c.VCHUNK : (nn + 1) * c.VCHUNK].rearrange(
                        "(ko p) n -> p ko n", p=P
                    ),
                )
                for st in range(ST):
                    pl = psum.tile([P, 512], F32, tag="mm", name="mm")[:, : c.VCHUNK]
                    for kk in range(ET):
                        nc.tensor.matmul(
                            pl,
                            lhsT=normT[:, kk, st * P : (st + 1) * P],
                            rhs=vw[:, kk, :],
                            start=(kk == 0),
                            stop=(kk == ET - 1),
                        )
                    lo = work.tile([P, c.VCHUNK], F32, tag="lo")
                    nc.vector.tensor_copy(out=lo[:], in_=pl)
                    nc.sync.dma_start(
                        out_d[st * P : (st + 1) * P,
                              nn * c.VCHUNK : (nn + 1) * c.VCHUNK],
                        lo[:],
                    )

    nc.compile()
    return nc


# ---------------- host side ----------------

def _rope_consts(cfg: Cfg):
    S, D = cfg.S, cfg.D
    half = D // 2
    i = np.arange(D)
    offset = i % half
    scales = np.power(10000.0, (-2.0 / D) * offset.astype(np.float32))
    m = np.arange(S, dtype=np.float32)
    angles = m[:, None] * scales[None, :]
    cos = np.cos(angles).astype(np.float32)
    sin = np.sin(angles).astype(np.float32)
    sin_eff = np.concatenate([-sin[:, :half], sin[:, half:]], axis=-1)
    qscale = 1.0 / math.sqrt(D)

    def to_tile(a):  # [S, D] -> [P, ST*D]
        return (
            a.reshape(cfg.ST, P, D).transpose(1, 0, 2).reshape(P, cfg.ST * D)
        )

    bf = ml_dtypes.bfloat16
    return (
        to_tile(cos * qscale).astype(bf),
        to_tile(sin_eff * qscale).astype(bf),
        to_tile(cos).astype(bf),
        to_tile(sin_eff).astype(bf),
    )


def make_in_maps(cfg: Cfg, tokens, table, qkv_w, out_w, up_w, down_w, vocab_w):
    c = cfg
    bf = ml_dtypes.bfloat16
    HD = c.HC * c.D        # head-dim cols per core
    H_ALL = c.n_cores * c.HC
    HID_ALL = c.n_cores * c.HIDC

    tokens = np.asarray(tokens).reshape(-1)
    idx = tokens.reshape(c.ST, P).T.astype(np.int32).copy()  # [P, ST]

    table = np.asarray(table, dtype=np.float32).copy()
    table[0] = 0.0
    table_bf = table.astype(bf)

    cosq, sinq, cosk, sink = _rope_consts(c)

    qkv_w = np.asarray(qkv_w, dtype=np.float32)
    out_w = np.asarray(out_w, dtype=np.float32)
    up_w = np.asarray(up_w, dtype=np.float32)
    down_w = np.asarray(down_w, dtype=np.float32)
    vocab_w = np.asarray(vocab_w, dtype=np.float32)

    in_maps = []
    for core in range(c.n_cores):
        hlo = core * HD
        q_cols = slice(hlo, hlo + HD)
        k_cols = slice(H_ALL * c.D + hlo, H_ALL * c.D + hlo + HD)
        v_cols = slice(2 * H_ALL * c.D + hlo, 2 * H_ALL * c.D + hlo + HD)
        qkv_c = np.concatenate(
            [qkv_w[:, :, q_cols], qkv_w[:, :, k_cols], qkv_w[:, :, v_cols]], axis=2
        ).astype(bf)
        out_c = out_w[:, hlo : hlo + HD, :].astype(bf)
        g_cols = slice(core * c.HIDC, (core + 1) * c.HIDC)
        u_cols = slice(HID_ALL + core * c.HIDC, HID_ALL + (core + 1) * c.HIDC)
        up_c = np.concatenate([up_w[:, :, g_cols], up_w[:, :, u_cols]], axis=2).astype(bf)
        down_c = down_w[:, core * c.HIDC : (core + 1) * c.HIDC, :].astype(bf)
        voc_c = vocab_w[:, core * c.VC : (core + 1) * c.VC].astype(bf)
        in_maps.append(
            {
                "idx": idx,
                "table": table_bf,
                "qkvw": np.ascontiguousarray(qkv_c),
                "outw": np.ascontiguousarray(out_c),
                "upw": np.ascontiguousarray(up_c),
                "downw": np.ascontiguousarray(down_c),
                "vocw": np.ascontiguousarray(voc_c),
                "cosq": cosq,
                "sinq": sinq,
                "cosk": cosk,
                "sink": sink,
            }
        )
    return in_maps


LAST_EXEC_TIME_NS = None
LAST_RESULTS = None


def kernel(tokens, table, qkv_w, out_w, up_w, down_w, vocab_w):
    global LAST_EXEC_TIME_NS, LAST_RESULTS
    cfg = Cfg()
    if os.environ.get("BASS_TRACE"):
        try:  # antenv.axon_hooks is missing in this image; provide it
            import types
            import antenv

            if "antenv.axon_hooks" not in sys.modules:
                mod = types.ModuleType("antenv.axon_hooks")
                mod._hook = None
                mod.set_axon_ntff_profile_hook = lambda h: setattr(mod, "_hook", h)
                mod.get_axon_ntff_profile_hook = lambda: mod._hook
                sys.modules["antenv.axon_hooks"] = mod
                antenv.axon_hooks = mod
                from trn_agent_boot.trn_boot import _ntff_profile_via_ctypes

                mod.set_axon_ntff_profile_hook(
                    _ntff_profile_via_ctypes("/opt/axon/libaxon_pjrt.so")
                )
        except Exception as e:
            print(f"[kernel] trace hook setup failed: {e}", file=sys.stderr)

    nc = build_kernel(cfg)
    in_maps = make_in_maps(cfg, tokens, table, qkv_w, out_w, up_w, down_w, vocab_w)
    res = run_bass_kernel_spmd(
        nc, in_maps, core_ids=list(range(cfg.n_cores)),
        trace=bool(os.environ.get("BASS_TRACE")),
    )
    LAST_EXEC_TIME_NS = res.exec_time_ns
    global LAST_RESULTS
    LAST_RESULTS = res
    logits = np.concatenate([r["out"] for r in res.results], axis=1)
    return logits[None].astype(np.float32)


# revision 27
# speedup vs baseline: 1.0168x; 1.0168x over previous
"""Trainium2 Bass kernel: 4-layer dense transformer (Megatron TP over 8 NeuronCores).

Sharding (per sharding hint):
  - qkv_w / up_w sharded along output dim, out_w / down_w along input dim.
  - Heads: 16 heads / 8 cores = 2 heads per core.
  - vocab_w sharded along vocab dim (4000 cols/core); host concatenates logits.
  - Norms + residual replicated; deltas AllReduced (bf16) after out-proj / down-proj.

Device layout strategy:
  - Residual `emb` kept in natural [s, e] layout (bf16), s-tiles of 128 on partitions.
  - Matmul inputs need e (contraction) on partitions -> normed activations are
    transposed via PE transpose; the rmsnorm scale is folded into the transpose by
    replacing the identity with diag(scale).
  - Attention computes S^T = K @ Q^T per head (k on partitions), exp via ACT,
    causal mask via gpsimd.affine_select on diagonal blocks, AV via
    matmul(lhsT=P^T, rhs=[V|1]) which yields O and the softmax denominator in
    one accumulation group.
"""

import os
import sys

if "/opt/trn_rl_repo" not in sys.path:
    sys.path.insert(0, "/opt/trn_rl_repo")

import math
from dataclasses import dataclass

import numpy as np
import ml_dtypes

import concourse.bass as bass
import concourse.bacc as bacc
import concourse.mybir as mybir
import concourse.tile as tile
from concourse.bass import IndirectOffsetOnAxis
from concourse.bass_utils import run_bass_kernel_spmd
from concourse.masks import make_identity

F32 = mybir.dt.float32
BF16 = mybir.dt.bfloat16
I32 = mybir.dt.int32
AF = mybir.ActivationFunctionType
OP = mybir.AluOpType

P = 128


@dataclass(frozen=True)
class Cfg:
    S: int = 2048        # sequence
    E: int = 1024        # embed
    D: int = 64          # head dim
    HC: int = 2          # heads per core
    HIDC: int = 512      # hidden shard per core (of 4096 / 8)
    VC: int = 4000       # vocab shard per core
    L: int = 4           # layers
    n_cores: int = 8
    V_ROWS: int = 32000  # embedding table rows
    eps: float = 1e-5

    @property
    def ST(self):
        return self.S // P

    @property
    def ET(self):
        return self.E // P

    @property
    def HT(self):
        return self.HIDC // P

    @property
    def QKVC(self):
        return 3 * self.HC * self.D

    @property
    def QC(self):
        return min(512, self.S)  # q chunk for attention

    @property
    def VCHUNK(self):
        for c in (500, 512, 256, 128):
            if self.VC % c == 0 and c <= 512:
                return c
        return self.VC


def build_kernel(cfg: Cfg):
    """Build the SPMD Bass graph. Returns the nc module."""
    c = cfg
    ST, ET, HT = c.ST, c.ET, c.HT
    D, HC = c.D, c.HC
    D2 = D // 2
    EG = min(4, ET)  # transpose group size (transposes per psum tile)

    nc = bacc.Bacc(
        "TRN2", target_bir_lowering=False, debug=False, num_devices=cfg.n_cores
    )

    # ---- DRAM parameters (per-core shards; same names in in_maps) ----
    idx_d = nc.declare_dram_parameter("idx", [P, ST], I32, isOutput=False)
    table_d = nc.declare_dram_parameter("table", [c.V_ROWS, c.E], BF16, isOutput=False)
    qkvw_d = nc.declare_dram_parameter("qkvw", [c.L, c.E, c.QKVC], BF16, isOutput=False)
    outw_d = nc.declare_dram_parameter("outw", [c.L, HC * D, c.E], BF16, isOutput=False)
    upw_d = nc.declare_dram_parameter("upw", [c.L, c.E, 2 * c.HIDC], BF16, isOutput=False)
    downw_d = nc.declare_dram_parameter("downw", [c.L, c.HIDC, c.E], BF16, isOutput=False)
    vocw_d = nc.declare_dram_parameter("vocw", [c.E, c.VC], BF16, isOutput=False)
    cos_d = nc.declare_dram_parameter("cos", [P, ST * D], BF16, isOutput=False)
    sin_d = nc.declare_dram_parameter("sin", [P, ST * D], BF16, isOutput=False)
    out_d = nc.declare_dram_parameter("out", [c.S, c.VC], F32, isOutput=True)

    from contextlib import ExitStack

    with tile.TileContext(nc) as tc, ExitStack() as es:
        const = es.enter_context(tc.tile_pool(name="const", bufs=1))
        wpool = es.enter_context(tc.tile_pool(name="w", bufs=1))
        act = es.enter_context(tc.tile_pool(name="act", bufs=1))
        act2 = es.enter_context(tc.tile_pool(name="act2", bufs=1))
        work = es.enter_context(tc.tile_pool(name="work", bufs=2))
        psum = es.enter_context(tc.tile_pool(name="psum", bufs=4, space="PSUM"))
        psum_tr = es.enter_context(tc.tile_pool(name="psumtr", bufs=2, space="PSUM"))
        dram = es.enter_context(tc.tile_pool(name="dram", bufs=8, space="DRAM"))
        if True:
            # ---- constants ----
            ident = const.tile([P, P], BF16, tag="ident")
            make_identity(nc, ident[:])
            rope_tiles = {}
            for name, d in (("cos", cos_d), ("sin", sin_d)):
                t = const.tile([P, ST, D], BF16, tag=name)
                nc.sync.dma_start(t[:], d[:].rearrange("p (st d) -> p st d", d=D))
                rope_tiles[name] = t
            idx_sb = const.tile([P, ST], I32, tag="idx")
            nc.sync.dma_start(idx_sb[:], idx_d[:])

            # ---- residual (bf16, natural [s-tile, e]) via embedding gather ----
            emb = act.tile([P, ST, c.E], BF16, tag="emb")
            for st in range(ST):
                nc.gpsimd.indirect_dma_start(
                    out=emb[:, st, :],
                    out_offset=None,
                    in_=table_d[:],
                    in_offset=IndirectOffsetOnAxis(ap=idx_sb[:, st : st + 1], axis=0),
                )

            def rmsnorm_T():
                """rmsnorm(emb) transposed -> [P, ET, S] bf16 (e on partitions)."""
                ssq = work.tile([P, ST], F32, tag="ssq")
                for st in range(ST):
                    scr = work.tile([P, c.E], BF16, tag="sqscr")
                    nc.scalar.activation(
                        out=scr[:], in_=emb[:, st, :], func=AF.Square,
                        accum_out=ssq[:, st : st + 1],
                    )
                ms = work.tile([P, ST], F32, tag="ms")
                nc.vector.tensor_scalar(
                    ms[:], ssq[:], 1.0 / c.E, c.eps, OP.mult, OP.add
                )
                rms = work.tile([P, ST], F32, tag="rms")
                nc.scalar.activation(out=rms[:], in_=ms[:], func=AF.Sqrt)
                scale = work.tile([P, ST], F32, tag="scale")
                nc.vector.reciprocal(out=scale[:], in_=rms[:])
                scale_b = work.tile([P, ST], BF16, tag="scaleb")
                nc.vector.tensor_copy(out=scale_b[:], in_=scale[:])

                normT = act2.tile([P, ET, c.S], BF16, tag="big")
                for st in range(ST):
                    diag = work.tile([P, P], BF16, tag="diag")
                    nc.gpsimd.affine_select(
                        out=diag[:],
                        in_=scale_b[:, st : st + 1].to_broadcast((P, P)),
                        pattern=[[-1, P]],
                        compare_op=OP.is_equal,
                        fill=0.0,
                        base=0,
                        channel_multiplier=1,
                    )
                    for eg in range(ET // EG):
                        ptr = psum_tr.tile([P, 512], F32, tag="tr", name="tr")[:, : EG * P]
                        for j in range(EG):
                            ee = eg * EG + j
                            nc.tensor.matmul(
                                ptr[:, j * P : (j + 1) * P],
                                lhsT=emb[:, st, ee * P : (ee + 1) * P],
                                rhs=diag[:],
                                start=True,
                                stop=True,
                            )
                        nc.vector.tensor_copy(
                            out=normT[:, eg * EG : (eg + 1) * EG,
                                      st * P : (st + 1) * P],
                            in_=ptr.rearrange("p (g q) -> p g q", g=EG),
                        )
                return normT

            GN = min(4, ST)          # AR pipeline groups
            GS = ST // GN            # s-tiles per group

            def ar_issue(delta_g):
                """delta_g: [P, GS, E] partial -> DMA to bounce + AllReduce.
                Returns the Shared output dram tile."""
                arin = dram.tile([GS * P, c.E], BF16, tag="arin", name="arin")
                arout = dram.tile(
                    [GS * P, c.E], BF16, tag="arout", name="arout",
                    addr_space="Shared" if c.n_cores > 4 else "Local",
                )
                nc.sync.dma_start(
                    arin[:].rearrange("(st p) e -> p st e", p=P), delta_g[:]
                )
                nc.gpsimd.collective_compute(
                    "AllReduce",
                    OP.add,
                    replica_groups=[list(range(c.n_cores))],
                    ins=[arin[:].opt()],
                    outs=[arout[:].opt()],
                )
                return arout

            def ar_collect(g, arout):
                gath = act2.tile([P, GS, c.E], BF16, tag="dg", name="gath", bufs=3)
                nc.sync.dma_start(
                    gath[:], arout[:].rearrange("(st p) e -> p st e", p=P)
                )
                nc.vector.tensor_tensor(
                    out=emb[:, g * GS : (g + 1) * GS, :],
                    in0=emb[:, g * GS : (g + 1) * GS, :],
                    in1=gath[:],
                    op=OP.add,
                )

            for l in range(c.L):
                # ---- layer weights (bf16, e/h on partitions) ----
                qkvw = wpool.tile([P, ET, c.QKVC], BF16, tag="qkvw", bufs=1)
                nc.sync.dma_start(
                    qkvw[:], qkvw_d[l].rearrange("(ko p) n -> p ko n", p=P)
                )
                outw = wpool.tile([P, c.E], BF16, tag="outw", bufs=1)
                nc.sync.dma_start(outw[:], outw_d[l])
                downw = wpool.tile([P, HT, c.E], BF16, tag="downw")
                nc.sync.dma_start(
                    downw[:], downw_d[l].rearrange("(ko p) n -> p ko n", p=P)
                )

                # ======== attention half ========
                normT = rmsnorm_T()

                # qkv (natural [s, 3*HC*D])
                qkvn = act.tile([P, ST, c.QKVC], BF16, tag="qg")
                for st in range(ST):
                    pq = psum.tile([P, 512], F32, tag="mm", name="mm")[:, : c.QKVC]
                    for kk in range(ET):
                        nc.tensor.matmul(
                            pq,
                            lhsT=normT[:, kk, st * P : (st + 1) * P],
                            rhs=qkvw[:, kk, :],
                            start=(kk == 0),
                            stop=(kk == ET - 1),
                        )
                    nc.vector.tensor_copy(out=qkvn[:, st, :], in_=pq)

                # RoPE (free-dim half swap; q also pre-scaled by 1/sqrt(D) via consts)
                rq = act.tile([P, ST, HC * D], BF16, tag="ra")
                rk = act.tile([P, ST, HC * D], BF16, tag="rb")
                for (src0, dst, cosw, sinw) in (
                    (0, rq, rope_tiles["cos"], rope_tiles["sin"]),
                    (HC * D, rk, rope_tiles["cos"], rope_tiles["sin"]),
                ):
                    for h in range(HC):
                        s0 = src0 + h * D
                        x = qkvn[:, :, s0 : s0 + D]
                        o = dst[:, :, h * D : (h + 1) * D]
                        tmp = work.tile([P, ST, D], BF16, tag="ropetmp")
                        nc.vector.tensor_tensor(
                            out=tmp[:, :, 0:D2],
                            in0=qkvn[:, :, s0 + D2 : s0 + D],
                            in1=sinw[:, :, 0:D2],
                            op=OP.mult,
                        )
                        nc.vector.tensor_tensor(
                            out=tmp[:, :, D2:D],
                            in0=qkvn[:, :, s0 : s0 + D2],
                            in1=sinw[:, :, D2:D],
                            op=OP.mult,
                        )
                        nc.vector.tensor_tensor(out=o, in0=x, in1=cosw[:], op=OP.mult)
                        nc.vector.tensor_tensor(out=o, in0=o, in1=tmp[:], op=OP.add)

                # V augmented with ones column per head: [.., h*(D+1)+64] = 1
                vaug = act.tile([P, ST, HC * (D + 1)], BF16, tag="vaug")
                nc.gpsimd.memset(vaug[:], 1.0)
                for h in range(HC):
                    nc.vector.tensor_copy(
                        out=vaug[:, :, h * (D + 1) : h * (D + 1) + D],
                        in_=qkvn[:, :, 2 * HC * D + h * D : 2 * HC * D + (h + 1) * D],
                    )

                # transpose rq/rk -> per-head [d, s], zero-padded to K=128
                rqT = act.tile([P, HC, c.S], BF16, tag="rqT")
                rkT = act.tile([P, HC, c.S], BF16, tag="rkT")
                nc.gpsimd.memset(rqT[D:P, :, :], 0.0)
                nc.gpsimd.memset(rkT[D:P, :, :], 0.0)
                for st in range(ST):
                    ptr = psum_tr.tile([P, 512], F32, tag="tr", name="tr")
                    for h in range(HC):
                        nc.tensor.matmul(
                            ptr[0:D, h * P : (h + 1) * P],
                            lhsT=rq[:, st, h * D : (h + 1) * D],
                            rhs=ident[:], start=True, stop=True)
                        nc.tensor.matmul(
                            ptr[0:D, (HC + h) * P : (HC + h + 1) * P],
                            lhsT=rk[:, st, h * D : (h + 1) * D],
                            rhs=ident[:], start=True, stop=True)
                    for h in range(HC):
                        nc.vector.tensor_copy(
                            out=rqT[0:D, h, st * P : (st + 1) * P],
                            in_=ptr[0:D, h * P : (h + 1) * P])
                        nc.vector.tensor_copy(
                            out=rkT[0:D, h, st * P : (st + 1) * P],
                            in_=ptr[0:D, (HC + h) * P : (HC + h + 1) * P])

                # scores + AV per head, causal, chunked over q
                atto = act.tile([P, ST, HC * D], BF16, tag="ra")
                n_chunks = c.S // c.QC
                for h in range(HC):
                    for ch in range(n_chunks):
                        q0 = ch * c.QC
                        nk = (q0 + c.QC) // P
                        PT = act2.tile([P, ST, c.QC], BF16, tag="big")
                        for kt in range(nk):
                            ps = psum.tile([P, 512], F32, tag="mm", name="mm")[:, : c.QC]
                            nc.tensor.matmul(
                                ps,
                                lhsT=rkT[:, h, kt * P : (kt + 1) * P],
                                rhs=rqT[:, h, q0 : q0 + c.QC],
                                start=True,
                                stop=True,
                            )
                            nc.scalar.activation(
                                out=PT[:, kt, :], in_=ps, func=AF.Exp
                            )
                            if (kt + 1) * P > q0:  # diagonal block: causal mask
                                nc.gpsimd.affine_select(
                                    out=PT[:, kt, :],
                                    in_=PT[:, kt, :],
                                    pattern=[[1, c.QC]],
                                    compare_op=OP.is_ge,
                                    fill=0.0,
                                    base=q0 - kt * P,
                                    channel_multiplier=-1,
                                )
                        for qs in range(c.QC // P):
                            po = psum.tile([P, 512], F32, tag="mm", name="mm")[:, : D + 1]
                            for kt in range(nk):
                                nc.tensor.matmul(
                                    po,
                                    lhsT=PT[:, kt, qs * P : (qs + 1) * P],
                                    rhs=vaug[:, kt, h * (D + 1) : (h + 1) * (D + 1)],
                                    start=(kt == 0),
                                    stop=(kt == nk - 1),
                                )
                            st_out = (q0 // P) + qs
                            rec = work.tile([P, 1], F32, tag="rec")
                            nc.vector.reciprocal(out=rec[:], in_=po[:, D : D + 1])
                            nc.vector.tensor_scalar_mul(
                                atto[:, st_out, h * D : (h + 1) * D],
                                po[:, 0:D], rec[:]
                            )

                # transpose attn out -> [hd, s]
                attoT = act.tile([P, c.S], BF16, tag="rb")
                for st in range(ST):
                    ptr = psum_tr.tile([P, 512], F32, tag="tr", name="tr")[:, :P]
                    nc.tensor.matmul(ptr, lhsT=atto[:, st, :], rhs=ident[:],
                                     start=True, stop=True)
                    nc.vector.tensor_copy(
                        out=attoT[:, st * P : (st + 1) * P], in_=ptr
                    )

                # out-proj (partial) -> grouped delta + pipelined AllReduce
                EW = min(512, c.E)
                arouts = []
                for g in range(GN):
                    delta_g = act2.tile([P, GS, c.E], BF16, tag="dg", name="dg", bufs=3)
                    for si in range(GS):
                        st = g * GS + si
                        for he in range(c.E // EW):
                            pd = psum.tile([P, 512], F32, tag="mm", name="mm")[:, :EW]
                            nc.tensor.matmul(
                                pd,
                                lhsT=attoT[:, st * P : (st + 1) * P],
                                rhs=outw[:, he * EW : (he + 1) * EW],
                                start=True,
                                stop=True,
                            )
                            nc.vector.tensor_copy(
                                out=delta_g[:, si, he * EW : (he + 1) * EW], in_=pd
                            )
                    arouts.append(ar_issue(delta_g))
                for g, aro in enumerate(arouts):
                    ar_collect(g, aro)

                # ======== FFN half ========
                normT = rmsnorm_T()
                gsil = act.tile([P, HT, c.S], BF16, tag="qg")
                hT = act.tile([P, HT, c.S], BF16, tag="hT")
                n_sc = c.S // 512 if c.S >= 512 else 1
                scw = c.S // n_sc
                for ct in range(2 * HT):
                    upw_ct = wpool.tile([P, ET, P], BF16, tag="upw", name="upw",
                                        bufs=2)
                    nc.sync.dma_start(
                        upw_ct[:],
                        upw_d[l][:, ct * P : (ct + 1) * P].rearrange(
                            "(ko p) n -> p ko n", p=P
                        ),
                    )
                    for sc in range(n_sc):
                        pu = psum.tile([P, 512], F32, tag="mm", name="mm")[:, :scw]
                        for kk in range(ET):
                            nc.tensor.matmul(
                                pu,
                                lhsT=upw_ct[:, kk, :],
                                rhs=normT[:, kk, sc * scw : (sc + 1) * scw],
                                start=(kk == 0),
                                stop=(kk == ET - 1),
                            )
                        if ct < HT:  # gate tiles: gsil = g * sigmoid(g)
                            sg = work.tile([P, scw], BF16, tag="sg")
                            nc.scalar.activation(out=sg[:], in_=pu, func=AF.Sigmoid)
                            nc.vector.tensor_tensor(
                                out=gsil[:, ct, sc * scw : (sc + 1) * scw],
                                in0=pu,
                                in1=sg[:],
                                op=OP.mult,
                            )
                        else:  # up tiles: h = up * gsil
                            nc.vector.tensor_tensor(
                                out=hT[:, ct - HT, sc * scw : (sc + 1) * scw],
                                in0=pu,
                                in1=gsil[:, ct - HT, sc * scw : (sc + 1) * scw],
                                op=OP.mult,
                            )

                arouts = []
                for g in range(GN):
                    delta_g = act2.tile([P, GS, c.E], BF16, tag="dg", name="dg", bufs=3)
                    for si in range(GS):
                        st = g * GS + si
                        for he in range(c.E // EW):
                            pd = psum.tile([P, 512], F32, tag="mm", name="mm")[:, :EW]
                            for kt in range(HT):
                                nc.tensor.matmul(
                                    pd,
                                    lhsT=hT[:, kt, st * P : (st + 1) * P],
                                    rhs=downw[:, kt, he * EW : (he + 1) * EW],
                                    start=(kt == 0),
                                    stop=(kt == HT - 1),
                                )
                            nc.vector.tensor_copy(
                                out=delta_g[:, si, he * EW : (he + 1) * EW], in_=pd
                            )
                    arouts.append(ar_issue(delta_g))
                for g, aro in enumerate(arouts):
                    ar_collect(g, aro)

            # ======== final norm + logits ========
            normT = rmsnorm_T()
            for nn in range(c.VC // c.VCHUNK):
                vw = wpool.tile([P, ET, c.VCHUNK], BF16, tag="vocw", bufs=2)
                nc.sync.dma_start(
                    vw[:],
                    vocw_d[:, nn * c.VCHUNK : (nn + 1) * c.VCHUNK].rearrange(
                        "(ko p) n -> p ko n", p=P
                    ),
                )
                for st in range(ST):
                    pl = psum.tile([P, 512], F32, tag="mm", name="mm")[:, : c.VCHUNK]
                    for kk in range(ET):
                        nc.tensor.matmul(
                            pl,
                            lhsT=normT[:, kk, st * P : (st + 1) * P],
                            rhs=vw[:, kk, :],
                            start=(kk == 0),
                            stop=(kk == ET - 1),
                        )
                    lo = work.tile([P, c.VCHUNK], F32, tag="lo")
                    nc.vector.tensor_copy(out=lo[:], in_=pl)
                    nc.sync.dma_start(
                        out_d[st * P : (st + 1) * P,
                              nn * c.VCHUNK : (nn + 1) * c.VCHUNK],
                        lo[:],
                    )

    nc.compile()
    return nc


# ---------------- host side ----------------

def _rope_consts(cfg: Cfg):
    S, D = cfg.S, cfg.D
    half = D // 2
    i = np.arange(D)
    offset = i % half
    scales = np.power(10000.0, (-2.0 / D) * offset.astype(np.float32))
    m = np.arange(S, dtype=np.float32)
    angles = m[:, None] * scales[None, :]
    cos = np.cos(angles).astype(np.float32)
    sin = np.sin(angles).astype(np.float32)
    sin_eff = np.concatenate([-sin[:, :half], sin[:, half:]], axis=-1)

    def to_tile(a):  # [S, D] -> [P, ST*D]
        return (
            a.reshape(cfg.ST, P, D).transpose(1, 0, 2).reshape(P, cfg.ST * D)
        )

    bf = ml_dtypes.bfloat16
    return to_tile(cos).astype(bf), to_tile(sin_eff).astype(bf)


def make_in_maps(cfg: Cfg, tokens, table, qkv_w, out_w, up_w, down_w, vocab_w):
    c = cfg
    bf = ml_dtypes.bfloat16
    HD = c.HC * c.D        # head-dim cols per core
    H_ALL = c.n_cores * c.HC
    HID_ALL = c.n_cores * c.HIDC

    tokens = np.asarray(tokens).reshape(-1)
    idx = tokens.reshape(c.ST, P).T.astype(np.int32).copy()  # [P, ST]

    table = np.asarray(table, dtype=np.float32).copy()
    table[0] = 0.0
    table_bf = table.astype(bf)

    cos_t, sin_t = _rope_consts(c)
    qscale = 1.0 / math.sqrt(c.D)

    qkv_w = np.asarray(qkv_w, dtype=np.float32)
    out_w = np.asarray(out_w, dtype=np.float32)
    up_w = np.asarray(up_w, dtype=np.float32)
    down_w = np.asarray(down_w, dtype=np.float32)
    vocab_w = np.asarray(vocab_w, dtype=np.float32)

    in_maps = []
    for core in range(c.n_cores):
        hlo = core * HD
        q_cols = slice(hlo, hlo + HD)
        k_cols = slice(H_ALL * c.D + hlo, H_ALL * c.D + hlo + HD)
        v_cols = slice(2 * H_ALL * c.D + hlo, 2 * H_ALL * c.D + hlo + HD)
        qkv_c = np.concatenate(
            [qkv_w[:, :, q_cols] * qscale, qkv_w[:, :, k_cols], qkv_w[:, :, v_cols]],
            axis=2,
        ).astype(bf)
        out_c = out_w[:, hlo : hlo + HD, :].astype(bf)
        g_cols = slice(core * c.HIDC, (core + 1) * c.HIDC)
        u_cols = slice(HID_ALL + core * c.HIDC, HID_ALL + (core + 1) * c.HIDC)
        up_c = np.concatenate([up_w[:, :, g_cols], up_w[:, :, u_cols]], axis=2).astype(bf)
        down_c = down_w[:, core * c.HIDC : (core + 1) * c.HIDC, :].astype(bf)
        voc_c = vocab_w[:, core * c.VC : (core + 1) * c.VC].astype(bf)
        in_maps.append(
            {
                "idx": idx,
                "table": table_bf,
                "qkvw": np.ascontiguousarray(qkv_c),
                "outw": np.ascontiguousarray(out_c),
                "upw": np.ascontiguousarray(up_c),
                "downw": np.ascontiguousarray(down_c),
                "vocw": np.ascontiguousarray(voc_c),
                "cos": cos_t,
                "sin": sin_t,
            }
        )
    return in_maps


LAST_EXEC_TIME_NS = None
LAST_RESULTS = None


def kernel(tokens, table, qkv_w, out_w, up_w, down_w, vocab_w):
    global LAST_EXEC_TIME_NS, LAST_RESULTS
    cfg = Cfg()
    if os.environ.get("BASS_TRACE"):
        try:  # antenv.axon_hooks is missing in this image; provide it
            import types
            import antenv

            if "antenv.axon_hooks" not in sys.modules:
                mod = types.ModuleType("antenv.axon_hooks")
                mod._hook = None
                mod.set_axon_ntff_profile_hook = lambda h: setattr(mod, "_hook", h)
                mod.get_axon_ntff_profile_hook = lambda: mod._hook
                sys.modules["antenv.axon_hooks"] = mod
                antenv.axon_hooks = mod
                from trn_agent_boot.trn_boot import _ntff_profile_via_ctypes

                mod.set_axon_ntff_profile_hook(
                    _ntff_profile_via_ctypes("/opt/axon/libaxon_pjrt.so")
                )
        except Exception as e:
            print(f"[kernel] trace hook setup failed: {e}", file=sys.stderr)

    nc = build_kernel(cfg)
    in_maps = make_in_maps(cfg, tokens, table, qkv_w, out_w, up_w, down_w, vocab_w)
    res = run_bass_kernel_spmd(
        nc, in_maps, core_ids=list(range(cfg.n_cores)),
        trace=bool(os.environ.get("BASS_TRACE")),
    )
    LAST_EXEC_TIME_NS = res.exec_time_ns
    global LAST_RESULTS
    LAST_RESULTS = res
    logits = np.concatenate([r["out"] for r in res.results], axis=1)
    return logits[None].astype(np.float32)
